# revision 1
# baseline (speedup 1.0000x reference)
"""Multi-head attention (B=8, N=1024, D=1024, H=16, Dh=64) on 8 TRN2 NeuronCores.

Sharding: pure data-parallel over batch — core i computes batch element i
end-to-end; weights are replicated. No collectives.

bf16 end-to-end: inputs are cast to bf16 on the HOST (numpy) so DMA traffic
halves and no on-device staging/convert is needed; the device output is
bf16 too (upcast to f32 on the host). rel err ~5.7e-3 vs the fp32
reference (gate is 2e-2).

Schedule (ACT's 128 exp tiles at ~1.04us each are the wall; PE is kept at
or under that rate everywhere):
  A: xT built by XBAR DMA-transpose straight from DRAM (14ns per 16x128
     tile; zero PE/DVE work), i=0:512 halves first so pair-0's q
     projection chases the per-dt transposes; q/k slab 0 is batched into
     2 multi-dim DMAs around them (per-DMA SEQ/HWDGE overhead is ~0.6us).
     Pair-0 scores+exp interleave 1:1 with the v-projection so ACT starts
     ~20us in while the v matmuls keep PE busy. v is stored
     [n, 16*(64+1)] with a ones column per head so the av matmul emits
     softmax denominators for free.
  B+D per head pair hp: proj_hp, scores_hp, av_{hp-1}. Scores feed ACT
     before av consumes the PREVIOUS pair's attn tiles (attn pool bufs=2),
     so ACT never starves at pair boundaries. The two heads of a pair are
     issued adjacently into PE row-groups 0:64 / 64:128 (concurrent on
     HW). av runs in [i, dv|den] layout — moving dim 65 at bf16 full rate
     (8.3k cycles/pair vs 16.4k column-wise), denominator in the same
     partition as its queries so normalize is a native per-partition
     tensor_scalar. mergedT for E is rebuilt per pair by SBUF->SBUF XBAR
     DMA-transposes (no PE/DVE).
  E: out = mergedT.T @ Wout + b_out; w_out prefetched in ONE batched DMA
     during pair 4 into the tile wv used in A; output staged through
     accumulator views carved from the dead q/k and slab tiles.
"""

import sys

sys.path.insert(0, "/opt/trn_rl_repo")

import numpy as np

B, N, DIM = 8, 1024, 1024
HEADS, DH = 16, 64
P = 128
T = N // P  # 8 tiles per 1024 dim
SCALE = DH**-0.5

_CACHE = {}


def _emit_body(nc, tc, tile, mybir, x_ext, wqkv_ext, wout_ext, bout_ext, out_ext, sfx):
    """Emit one full attention pass reading x_ext (bf16), writing out_ext."""
    F32 = mybir.dt.float32
    BF16 = mybir.dt.bfloat16
    Exp = mybir.ActivationFunctionType.Exp
    mult = mybir.AluOpType.mult
    add = mybir.AluOpType.add

    CHUNKS = [(0, 512), (512, 512)]  # matmul moving size is ISA-capped at 512
    ODT = out_ext.dtype

    with (
        tc.tile_pool(name=f"const{sfx}", bufs=1) as const,
        tc.tile_pool(name=f"merged{sfx}", bufs=1) as merged_pool,
        tc.tile_pool(name=f"xt{sfx}", bufs=1) as xt_pool,
        tc.tile_pool(name=f"vp{sfx}", bufs=1) as v_pool,
        tc.tile_pool(name=f"qk{sfx}", bufs=1) as qkp,
        tc.tile_pool(name=f"wqk{sfx}", bufs=2) as wqkp,
        tc.tile_pool(name=f"attn{sfx}", bufs=2) as attnp,
        tc.tile_pool(name=f"wo{sfx}", bufs=1) as woutp,
        tc.tile_pool(name=f"outp{sfx}", bufs=2) as outp,
        tc.tile_pool(name=f"small{sfx}", bufs=2) as small,
        tc.tile_pool(name=f"rcp{sfx}", bufs=6) as rcp,
    ):
        # warm the ACT exp table set at t=0
        wsrc = const.tile([1, 1], F32, tag="wsrc", name=f"wsrc{sfx}")
        nc.gpsimd.memset(wsrc[:], 0.0)
        warm = const.tile([1, 1], F32, tag="warm", name=f"warm{sfx}")
        nc.scalar.activation(warm[:], wsrc[:], Exp)

        mergedT = [
            merged_pool.tile([P, N], BF16, tag=f"m{a}", name=f"m{a}{sfx}")
            for a in range(T)
        ]
        merged_i = [
            merged_pool.tile([P, N], BF16, tag=f"mi{t}", name=f"mi{t}{sfx}")
            for t in range(T)
        ]
        xT = [
            xt_pool.tile([P, N], BF16, tag=f"xT{t}", name=f"xT{t}{sfx}")
            for t in range(T)
        ]
        # flat [128, 16*65]; head h's [v | ones] block is cols h*65..h*65+65
        v = [
            v_pool.tile([P, HEADS * (DH + 1)], BF16, tag=f"v{t}", name=f"v{t}{sfx}")
            for t in range(T)
        ]

        def load_w(pool, dram_ap, cols, tag):
            w_sb = pool.tile([P, cols], BF16, tag=tag, name=f"{tag}{sfx}")
            nc.sync.dma_start(w_sb[:], dram_ap)
            return w_sb

        def load_slab_half(k, s, out):
            # one q/k half of quarter-slab k as ONE batched DMA:
            # DRAM [8dt x 128 x 256] -> SBUF [128, 8dt*256]
            # (per-DMA-instruction SEQ/HWDGE overhead is ~0.6us, so batch).
            base = k * 256 + s * DIM
            w_sb = wqkp.tile([P, T * 256], BF16, tag=f"ws{s}", name=f"ws{s}{sfx}")
            nc.sync.dma_start(
                w_sb[:].rearrange("p (dt c) -> p dt c", c=256),
                wqkv_ext[:, base : base + 256].rearrange("(dt p) c -> p dt c", p=P),
            )
            out.append([w_sb[:, dt * 256 : (dt + 1) * 256] for dt in range(T)])
            return out

        def load_slab(k, split=False):
            out = []
            load_slab_half(k, 0, out)
            if not split:
                load_slab_half(k, 1, out)
            return out

        def proj_half(w_tiles, col, dst, c, w, psum_pool, tag):
            """One 512-wide projection chunk: dst[:, c:c+w] = (w.T @ xT)[…]."""
            ps = psum_pool.tile([P, 512], F32, tag=tag, name=f"pp{sfx}")
            for dt in range(T):
                nc.tensor.matmul(
                    ps[:],
                    w_tiles[dt][:, col * P : (col + 1) * P],
                    xT[dt][:, c : c + w],
                    start=(dt == 0),
                    stop=(dt == T - 1),
                )
            nc.vector.tensor_copy(dst[:, c : c + w], ps[:])

        def emit_score_pair(jt, q_sb, k_sb, pss):
            """scores + exp for j-tile jt, both heads (adjacent row-groups)."""
            out = []
            for sub in range(2):
                ro = sub * DH
                ps_s = pss.tile([P, N], F32, tag="pss", name=f"pss{sfx}")
                for c, w in CHUNKS:
                    nc.tensor.matmul(
                        ps_s[:, c : c + w],
                        k_sb[ro : ro + DH, jt * P : (jt + 1) * P],
                        q_sb[ro : ro + DH, c : c + w],
                        start=True,
                        stop=True,
                    )
                at_sb = attnp.tile(
                    [P, N], BF16, tag=f"at{jt}_{sub}", name=f"at{jt}_{sub}{sfx}"
                )
                nc.scalar.activation(at_sb[:], ps_s[:], Exp, scale=SCALE)
                out.append(at_sb)
            return out

        def emit_av_block(hp, attn_tiles, sub, it, psav, wide=False):
            """attn @ v for head 2*hp+sub, i-block it, in [i, dv] layout:
            out[i, dv|den] = sum_j attnT[j,i].T @ [v|1][j,dv] — moving dim 65
            at bf16 full rate (8.3k cycles/pair vs 16.4k the other way).
            The softmax denominator lands in column 64 of the SAME partition
            as its queries, so normalize is a native per-partition
            tensor_scalar — no partition_broadcast needed."""
            h = 2 * hp + sub
            if wide:
                ps_b = psav.tile([P, 512], F32, tag="psqk", name=f"psb{sfx}")[
                    :, 0 : DH + 1
                ]
            else:
                ps_b = psav.tile([P, DH + 1], F32, tag="psb", name=f"psb{sfx}")
            for jt in range(T):
                nc.tensor.matmul(
                    ps_b,
                    attn_tiles[sub][jt][:, it * P : (it + 1) * P],
                    v[jt][:, h * (DH + 1) : (h + 1) * (DH + 1)],
                    start=(jt == 0),
                    stop=(jt == T - 1),
                )
            rc = rcp.tile([P, 1], F32, tag="rc", name=f"rc{sfx}")
            nc.vector.reciprocal(rc[:], ps_b[:, DH : DH + 1])
            nc.vector.tensor_scalar(
                merged_i[it][:, h * DH : (h + 1) * DH],
                ps_b[:, 0:DH],
                rc[:, 0:1],
                None,
                mult,
            )

        def emit_av_pair(hp, attn_tiles, psavA, psavB):
            for it in range(T):
                emit_av_block(hp, attn_tiles, 0, it, psavA)
                emit_av_block(hp, attn_tiles, 1, it, psavB)

        def emit_mergedT(hp):
            # mergedT[hp] via SBUF->SBUF XBAR DMA transpose (no PE/DVE)
            for it in range(T):
                nc.sync.dma_start_transpose(
                    mergedT[hp][:, it * P : (it + 1) * P],
                    merged_i[it][:, hp * P : (hp + 1) * P],
                )

        # ---- Phase A: XBAR-transposed x + pair-0 proj/scores + C ----
        with (
            tc.tile_pool(name=f"psv{sfx}", bufs=2, space="PSUM") as psv,
            tc.tile_pool(name=f"pssA{sfx}", bufs=3, space="PSUM") as pssA,
        ):
            # DMA queue order: q-half of slab 0 first (1.5us), then the
            # i=0:512 half of every xT dim-block — pair-0's q projection
            # matmuls chase the per-dt transposes as each lands — then the
            # k-half, the remaining xT halves, wv, bias.
            slabs = {0: load_slab(0, split=True)}
            for dt in range(T):
                nc.sync.dma_start_transpose(
                    xT[dt][:, 0:512], x_ext[0:512, dt * P : (dt + 1) * P]
                )
            load_slab_half(0, 1, slabs[0])
            for dt in range(T):
                nc.sync.dma_start_transpose(
                    xT[dt][:, 512:1024], x_ext[512:1024, dt * P : (dt + 1) * P]
                )
            # wv in ONE batched DMA into the tile w_out will reuse later
            wv_sb = woutp.tile([P, T * DIM], BF16, tag="wo", name=f"wv{sfx}")
            nc.sync.dma_start(
                wv_sb[:].rearrange("p (dt c) -> p dt c", c=DIM),
                wqkv_ext[:, 2 * DIM : 3 * DIM].rearrange("(dt p) c -> p dt c", p=P),
            )
            wv = [wv_sb[:, dt * DIM : (dt + 1) * DIM] for dt in range(T)]
            b_sb = small.tile([1, DIM], BF16, tag="b_sb", name=f"b_sb{sfx}")
            nc.sync.dma_start(b_sb[:], bout_ext[:])
            b_bcast = outp.tile([P, DIM], BF16, tag="b_bcast", name=f"b_bcast{sfx}")
            nc.gpsimd.partition_broadcast(b_bcast[:], b_sb[:])

            def emit_C(k):
                # v-projection for x row k
                nc.gpsimd.memset(v[k][:], 1.0)
                for c, w in CHUNKS:
                    ps = psv.tile([P, 512], F32, tag="psv", name=f"psv{sfx}")
                    for dt in range(T):
                        nc.tensor.matmul(
                            ps[:],
                            xT[dt][:, k * P : (k + 1) * P],
                            wv[dt][:, c : c + w],
                            start=(dt == 0),
                            stop=(dt == T - 1),
                        )
                    nc.vector.tensor_copy(
                        v[k][:].rearrange("p (h c) -> p h c", c=DH + 1)[
                            :, (c // DH) : (c // DH) + 8, 0:DH
                        ],
                        ps[:].rearrange("p (h c) -> p h c", c=DH),
                    )

            # pair-0 q/k projection (chunk 0 needs only the first xT halves)
            q_sb = qkp.tile([P, N], BF16, tag="q0", name=f"q0{sfx}")
            k_sb = qkp.tile([P, N], BF16, tag="k0", name=f"k0{sfx}")
            wq0, wk0 = slabs[0]
            proj_half(wq0, 0, q_sb, 0, 512, psv, "psv")
            proj_half(wk0, 0, k_sb, 0, 512, psv, "psv")
            proj_half(wq0, 0, q_sb, 512, 512, psv, "psv")
            proj_half(wk0, 0, k_sb, 512, 512, psv, "psv")

            # pair-0 scores interleaved with the v-projection: ACT starts
            # exp'ing ~20us in while the v matmuls keep PE busy
            attn0 = [[], []]
            for jt in range(T):
                s0, s1 = emit_score_pair(jt, q_sb, k_sb, pssA)
                attn0[0].append(s0)
                attn0[1].append(s1)
                emit_C(jt)

        # ---- Phases B+D pipelined per head pair; E split into the tail:
        # head groups 0..3 are PSUM-chained into SBUF accumulators during
        # pair 7 (hiding inside its ~13us exp window), the 4..7 remainder
        # runs after av7 with at=7 last so the mergedT[7] DMA-transpose
        # overlaps the chain. Accumulators are carved out of the dead q/k
        # and slab tiles (zero extra SBUF). ----
        with (
            tc.tile_pool(name=f"psqk{sfx}", bufs=2, space="PSUM") as psqk,
            tc.tile_pool(name=f"pss{sfx}", bufs=2, space="PSUM") as pss,
            tc.tile_pool(name=f"psavA{sfx}", bufs=1, space="PSUM") as psavA,
            tc.tile_pool(name=f"psavB{sfx}", bufs=1, space="PSUM") as psavB,
        ):
            prev_attn = attn0
            wout_tiles = {}
            for hp in range(1, 8):
                if hp in (1, 3, 5):
                    slabs[(hp + 1) // 2] = load_slab((hp + 1) // 2)
                wq, wk = slabs[hp // 2]
                q_sb = qkp.tile([P, N], BF16, tag=f"q{hp % 2}", name=f"q{hp}{sfx}")
                k_sb = qkp.tile([P, N], BF16, tag=f"k{hp % 2}", name=f"k{hp}{sfx}")
                col = hp % 2
                proj_half(wq, col, q_sb, 0, 512, psqk, "psqk")
                proj_half(wq, col, q_sb, 512, 512, psqk, "psqk")
                proj_half(wk, col, k_sb, 0, 512, psqk, "psqk")
                proj_half(wk, col, k_sb, 512, 512, psqk, "psqk")
                attn_tiles = [[], []]
                for jt in range(T):
                    s0, s1 = emit_score_pair(jt, q_sb, k_sb, pss)
                    attn_tiles[0].append(s0)
                    attn_tiles[1].append(s1)
                emit_av_pair(hp - 1, prev_attn, psavA, psavB)
                emit_mergedT(hp - 1)
                prev_attn = attn_tiles
                # w_out prefetch in one batched DMA (reuses the wv tile)
                if hp == 4:
                    wo_sb = woutp.tile([P, T * DIM], BF16, tag="wo", name=f"wo{sfx}")
                    nc.sync.dma_start(
                        wo_sb[:].rearrange("p (dt c) -> p dt c", c=DIM),
                        wout_ext[:].rearrange("(dt p) c -> p dt c", p=P),
                    )
                    for at in range(T):
                        wout_tiles[at] = wo_sb[:, at * DIM : (at + 1) * DIM]

            # output staging carved from dead q/k tiles (nt 0..3) + slab
            # tiles (4..7) — zero extra SBUF
            acc = []
            for tag in ("q0", "k0", "q1", "k1"):
                acc.append(
                    qkp.tile([P, N], BF16, tag=tag, name=f"acc{tag}{sfx}")[:]
                )
            for s in range(2):
                big = wqkp.tile(
                    [P, T * 256], BF16, tag=f"ws{s}", name=f"accw{s}{sfx}"
                )
                acc.append(big[:, 0:N])
                acc.append(big[:, N : 2 * N])

            emit_av_pair(7, prev_attn, psavA, psavB)
            emit_mergedT(7)

        # ---- Phase E: out = mergedT.T @ Wout + b_out ----
        with tc.tile_pool(name=f"psf{sfx}", bufs=3, space="PSUM") as psf:
            for nt in range(T):
                ps = psf.tile([P, DIM], F32, tag="psf", name=f"psf{sfx}")
                for at in range(T):
                    lhsT = mergedT[at][:, nt * P : (nt + 1) * P]
                    for c, w in CHUNKS:
                        nc.tensor.matmul(
                            ps[:, c : c + w],
                            lhsT,
                            wout_tiles[at][:, c : c + w],
                            start=(at == 0),
                            stop=(at == T - 1),
                        )
                nc.vector.tensor_tensor(acc[nt], ps[:], b_bcast[:], add)
                nc.sync.dma_start(out_ext[nt * P : (nt + 1) * P, :], acc[nt])

def _build(reps=1, variant=None):
    import concourse.tile as tile
    from concourse import bacc, mybir

    F32 = mybir.dt.float32
    BF16 = mybir.dt.bfloat16

    nc = bacc.Bacc("TRN2", target_bir_lowering=False, debug=False, num_devices=8)
    x_ext = nc.declare_dram_parameter("x", [N, DIM], BF16, isOutput=False)
    wqkv_ext = nc.declare_dram_parameter("w_qkv", [DIM, 3 * DIM], BF16, isOutput=False)
    wout_ext = nc.declare_dram_parameter("w_out", [DIM, DIM], BF16, isOutput=False)
    bout_ext = nc.declare_dram_parameter("b_out", [1, DIM], BF16, isOutput=False)
    out_ext = nc.declare_dram_parameter("out", [N, DIM], BF16, isOutput=True)
    bounce = [
        nc.dram_tensor(f"bounce{k}", [N, DIM], BF16) for k in range(max(0, reps - 1))
    ]

    with tile.TileContext(nc) as tc:
        for k in range(reps):
            src = x_ext if k == 0 else bounce[k - 1]
            dst = out_ext if k == reps - 1 else bounce[k]
            _emit_body(
                nc, tc, tile, mybir, src, wqkv_ext, wout_ext, bout_ext, dst, f"_{k}"
            )
    nc.compile()
    return nc


def _get_nc(reps=1, variant=None):
    key = ("nc", reps)
    if key not in _CACHE:
        _CACHE[key] = _build(reps)
    return _CACHE[key]


def run(inputs, trace=False, reps=1, variant=None):
    import ml_dtypes
    from concourse.bass_utils import run_bass_kernel_spmd

    BF = ml_dtypes.bfloat16
    nc = _get_nc(reps)
    x = np.ascontiguousarray(np.asarray(inputs["x"]).astype(BF))
    w_qkv = np.ascontiguousarray(np.asarray(inputs["w_qkv"]).astype(BF))
    w_out = np.ascontiguousarray(np.asarray(inputs["w_out"]).astype(BF))
    b_out = np.ascontiguousarray(np.asarray(inputs["b_out"]).astype(BF)).reshape(1, DIM)
    in_maps = [
        {"x": x[i], "w_qkv": w_qkv, "w_out": w_out, "b_out": b_out} for i in range(B)
    ]
    res = run_bass_kernel_spmd(nc, in_maps, core_ids=list(range(B)), trace=trace)
    out = np.stack([res.results[i]["out"] for i in range(B)]).astype(np.float32)
    return out, res


def kernel(**inputs) -> np.ndarray:
    out, _ = run(inputs)
    return out

